# revision 52
# baseline (speedup 1.0000x reference)
"""Trainium2 Bass kernel for a ConvViT-style dense transformer block.

Reference computation (B=2, N=3136=56x56, C=512, 8 heads, hidden 2048):
    x = x + Attn(LN1(x));  x = x + MLP(LN2(x))
    MLP = fc2(gelu(dwconv3x3(fc1(.)) + dw_b))

Sharding: tokens are sharded 8 ways as (batch, 14-image-row) stripes.
Each core computes attention/MLP for its own 14 rows (plus 1 halo row on
each side for the depthwise conv), recomputing K/V projections for its
full batch locally (no collectives).  Host does the (free) scatter/gather.

Key engine assignment:
  - scores/PV/projections/fc1/fc2 and the 3x3 depthwise conv (as 9
    accumulating diagonal matmuls) run on the PE.
  - softmax exp runs on ACT as fused multi-bank [128, ~1536] activations.
  - LN applies run on ACT (Identity with per-partition scale/bias).
  - softmax 1/sum is folded into the PSUM->SBUF copy (DVE reciprocal +
    DMA partition-broadcast).
"""

import numpy as np

# ---------------- problem constants (hardcoded per spec) ----------------
B = 2
HI = 56          # image rows
WI = 56          # image cols
NB = HI * WI     # tokens per batch = 3136
C = 512
NH = 8
HD = 64
F3 = 3 * C       # 1536
HID = 4 * C      # 2048
EPS = 1e-5
NCORES = 8
RPC = HI // 4    # image rows per core = 14
EXTR = RPC + 2   # rows incl halo = 16
EXT = EXTR * WI  # 896 ext tokens
OWN = RPC * WI   # 784 own tokens
QCHS = [(0, 512), (512, 384)]       # attention q-chunks (bank-aligned)
FCHS = [(0, 504), (504, 392)]       # fc1 chunks: rows 0-8 / 9-15

_CACHE = {}


def _btiles():
    # 128-token tiles over the full batch (24 x 128 + 1 x 64)
    return [(i * 128, min(128, NB - i * 128)) for i in range((NB + 127) // 128)]


def _bchunks():
    # 512-token chunks over the full batch (6 x 512 + 1 x 64)
    return [(i * 512, min(512, NB - i * 512)) for i in range((NB + 511) // 512)]


def _build_nc():
    import concourse.bass as bass
    import concourse.bacc as bacc
    import concourse.tile as tile
    from concourse import mybir

    f32 = mybir.dt.float32
    b16 = mybir.dt.bfloat16
    AF = mybir.ActivationFunctionType
    OP = mybir.AluOpType

    nc = bacc.Bacc(trn_type="TRN2")

    # ---- external I/O ----
    xb_d = nc.dram_tensor("xb", [NB, C], f32, kind="ExternalInput")
    xe_d = nc.dram_tensor("xe", [EXT, C], f32, kind="ExternalInput")
    xeb_d = nc.dram_tensor("xeb", [EXT, C], f32, kind="ExternalInput")
    mask_d = nc.dram_tensor("mask", [EXT], b16, kind="ExternalInput")
    qkvT_d = nc.dram_tensor("qkvT", [C, F3], b16, kind="ExternalInput")
    qkvbc_d = nc.dram_tensor("qkvbc", [128, 8], f32, kind="ExternalInput")
    vbias_d = nc.dram_tensor("vbias", [1, C], f32, kind="ExternalInput")
    outT_d = nc.dram_tensor("outT", [C, C], b16, kind="ExternalInput")
    fc1T_d = nc.dram_tensor("fc1T", [C, HID], b16, kind="ExternalInput")
    fc1bg_d = nc.dram_tensor("fc1bg", [128, 16], f32, kind="ExternalInput")
    fc2T_d = nc.dram_tensor("fc2T", [HID, C], b16, kind="ExternalInput")
    fc2b_d = nc.dram_tensor("fc2b", [1, C], b16, kind="ExternalInput")
    dww_d = nc.dram_tensor("dww", [HID, 9], f32, kind="ExternalInput")
    dwb_d = nc.dram_tensor("dwb", [HID], f32, kind="ExternalInput")
    ident_d = nc.dram_tensor("ident", [128, 128], b16, kind="ExternalInput")
    out_d = nc.dram_tensor("out", [OWN, C], f32, kind="ExternalOutput")
    # scratch for the softmax-sum reciprocal roundtrip (reshape to 128 lanes
    # and broadcast back; FIFO order on the sync DMA queue serializes RAW)
    sums_d = nc.dram_tensor("sums_scr", [8, 512], b16, kind="Internal")
    rsum_d = nc.dram_tensor("rsum_scr", [8, 512], b16, kind="Internal")

    btiles = _btiles()
    bchunks = _bchunks()
    etiles = [(i * 128, 128) for i in range(EXT // 128)]          # 7 x 128
    otiles = [(i * 128, min(128, OWN - i * 128)) for i in range((OWN + 127) // 128)]

    with tile.TileContext(nc) as tc:
        from contextlib import ExitStack

        with ExitStack() as ctx:
            wp = ctx.enter_context(tc.tile_pool(name="wp", bufs=1))
            big = ctx.enter_context(tc.tile_pool(name="big", bufs=1))
            stage = ctx.enter_context(tc.tile_pool(name="stage", bufs=3))
            small = ctx.enter_context(tc.tile_pool(name="small", bufs=4))
            exr = ctx.enter_context(tc.tile_pool(name="exr", bufs=2))
            padp = ctx.enter_context(tc.tile_pool(name="padp", bufs=2))
            dgp = ctx.enter_context(tc.tile_pool(name="dgp", bufs=2))
            # PSUM: score-group ring 2x3 banks + o accumulators 2x1 bank
            psg = ctx.enter_context(tc.tile_pool(name="psg", bufs=2, space="PSUM"))
            pso = ctx.enter_context(tc.tile_pool(name="pso", bufs=1, space="PSUM"))
            _ps_ctr = [0]

            def mk_ps():
                # general-purpose [128,512] psum (projections, out-proj, fc1,
                # fc2, conv) — rotates over the score-group ring's banks,
                # which are free outside the attention inner loop.
                g = psg.tile([128, 1536], f32, tag="sg", name="ps_g")
                return g[:, 0:512]

            # ---------------- constants / weights into SBUF ----------------
            qkvT = wp.tile([128, 4, F3], b16, tag="qkvT")
            nc.sync.dma_start(out=qkvT, in_=qkvT_d[:, :].rearrange("(g p) f -> p g f", p=128))
            qkvbc = wp.tile([128, 8], f32, tag="qkvbc")
            nc.sync.dma_start(out=qkvbc, in_=qkvbc_d[:, :])
            vbias_sb = wp.tile([128, C], b16, tag="vbias")
            nc.gpsimd.dma_start(
                out=vbias_sb,
                in_=bass.AP(tensor=vbias_d[:, :].tensor, offset=0, ap=[[0, 128], [1, C]]),
            )
            outTs = wp.tile([64, 8, C], b16, tag="outTs")
            nc.sync.dma_start(out=outTs, in_=outT_d[:, :].rearrange("(h p) f -> p h f", p=64))
            fc1T = wp.tile([128, 4, HID], b16, tag="fc1T")
            nc.sync.dma_start(out=fc1T, in_=fc1T_d[:, :].rearrange("(g p) f -> p g f", p=128))
            fc1bg = wp.tile([128, 16], f32, tag="fc1bg")
            nc.sync.dma_start(out=fc1bg, in_=fc1bg_d[:, :])
            fc2b = wp.tile([1, C], b16, tag="fc2b")
            nc.sync.dma_start(out=fc2b, in_=fc2b_d[:, :])
            dww = wp.tile([128, 16, 9], f32, tag="dww")
            nc.sync.dma_start(out=dww, in_=dww_d[:, :].rearrange("(g p) t -> p g t", p=128))
            dwb = wp.tile([128, 16], f32, tag="dwb")
            nc.sync.dma_start(out=dwb, in_=dwb_d[:].rearrange("(g p) -> p g", p=128))
            maskb = wp.tile([128, EXT], b16, tag="maskb")
            nc.sync.dma_start(
                out=maskb,
                in_=bass.AP(tensor=mask_d[:].tensor, offset=0, ap=[[0, 128], [1, EXT]]),
            )
            ones = wp.tile([1, C], b16, tag="ones")
            nc.vector.memset(ones, 1.0)
            epsc = wp.tile([128, 1], f32, tag="epsc")
            nc.vector.memset(epsc, EPS)
            ident = wp.tile([128, 128], b16, tag="ident")
            nc.sync.dma_start(out=ident, in_=ident_d[:, :])

            # ---------------- LN helpers ----------------
            def layer_norm_tile(xt, ts, lt):
                # stats on DVE; rstd via ACT sqrt + DVE reciprocal;
                # apply on ACT (Identity with per-partition scale/bias)
                st = small.tile([128, 6], f32, tag="st")
                nc.vector.bn_stats(out=st[:ts], in_=xt[:ts])
                mv = small.tile([128, 4], f32, tag="mv")
                nc.vector.bn_aggr(out=mv[:ts, 0:2], in_=st[:ts])
                nc.scalar.activation(
                    out=mv[:ts, 1:2], in_=mv[:ts, 1:2], func=AF.Sqrt,
                    bias=epsc[:ts], scale=1.0,
                )
                nc.vector.reciprocal(out=mv[:ts, 1:2], in_=mv[:ts, 1:2])
                # mv[:,2] = -mu * rstd
                # mv[:,2] = -mu * rstd; apply on ACT (Identity w/ scale+bias)
                nc.vector.scalar_tensor_tensor(
                    out=mv[:ts, 2:3], in0=mv[:ts, 0:1], scalar=-1.0,
                    in1=mv[:ts, 1:2], op0=OP.mult, op1=OP.mult,
                )
                nc.scalar.activation(
                    out=lt[:ts], in_=xt[:ts], func=AF.Identity,
                    bias=mv[:ts, 2:3], scale=mv[:ts, 1:2],
                )

            def dma_transpose(lt, ts, t0, dest):
                # PE-transpose [ts,128] blocks into c-major storage, copied
                # out by DVE (ACT is the global gate; xbar DMA-transpose
                # measured ~1.2us/block and saturates the sync queue).
                for cc in range(4):
                    tp = psg.tile([128, 128], b16, tag="sg", name="tp")
                    nc.tensor.transpose(
                        tp[:, :ts], lt[:ts, cc * 128 : (cc + 1) * 128], ident[:ts, :ts]
                    )
                    if cc % 2 == 0:
                        nc.vector.tensor_copy(out=dest(cc, t0, ts), in_=tp[:, :ts])
                    else:
                        nc.scalar.activation(
                            out=dest(cc, t0, ts), in_=tp[:, :ts], func=AF.Copy
                        )

            # ---------------- LN1 + K/V projections, interleaved ----------------
            ln1xT = [big.tile([128, NB], b16, tag=f"lx{c}", name=f"ln1xT{c}") for c in range(4)]
            ln1eT = big.tile([128, 4, EXT], b16, tag="le")
            KT = [big.tile([128, NB], b16, tag=f"kt{c}", name=f"KT{c}") for c in range(4)]
            V5 = big.tile([128, len(btiles), 8, 65], b16, tag="v5")
            nc.vector.memset(V5[:, :, :, 64:65], 1.0)

            def put_ln1x(cc, t0, ts):
                return ln1xT[cc][:, t0 : t0 + ts]

            def put_ln1e(cc, t0, ts):
                return ln1eT[:, cc, t0 : t0 + ts]

            def ln1_tile(src_d, t0, ts, put, idx=None):
                xt = stage.tile([128, C], f32, tag="xf")
                nc.gpsimd.dma_start(out=xt[:ts], in_=src_d[t0 : t0 + ts, :])
                lt = stage.tile([128, C], b16, tag="xl")
                layer_norm_tile(xt, ts, lt)
                dma_transpose(lt, ts, t0, put)

            def v_proj_tile(i, t0, ts):
                # V bias is folded in via the broadcast vbias tile (free-dim
                # varying, so it rides the DVE copy as an STT add)
                ps = mk_ps()
                for c in range(4):
                    nc.tensor.matmul(
                        ps[:ts],
                        ln1xT[c][:, t0 : t0 + ts],
                        qkvT[:, c, 2 * C : 3 * C],
                        start=(c == 0), stop=(c == 3),
                    )
                nc.vector.scalar_tensor_tensor(
                    out=V5[:ts, i, :, 0:64],
                    in0=ps[:ts].rearrange("p (h d) -> p h d", d=64),
                    scalar=1.0,
                    in1=vbias_sb[:ts].rearrange("p (h d) -> p h d", d=64),
                    op0=OP.bypass, op1=OP.add,
                )

            def k_proj_f(f, t0, tn):
                # K bias folded into the ACT copy (per-partition Identity bias)
                ps = mk_ps()
                for c in range(4):
                    nc.tensor.matmul(
                        ps[:, :tn],
                        qkvT[:, c, C + f * 128 : C + (f + 1) * 128],
                        ln1xT[c][:, t0 : t0 + tn],
                        start=(c == 0), stop=(c == 3),
                    )
                nc.scalar.activation(
                    out=KT[f][:, t0 : t0 + tn], in_=ps[:, :tn],
                    func=AF.Identity, bias=qkvbc[:, f : f + 1], scale=1.0,
                )

            QT = big.tile([128, 4, EXT], b16, tag="qt")

            def q_proj_f(f, q0, qn, on_act=True):
                # Q bias folded into the copy (ACT in the head; DVE when
                # emitted inside the attention exp window)
                ps = mk_ps()
                for c in range(4):
                    nc.tensor.matmul(
                        ps[:, :qn],
                        qkvT[:, c, f * 128 : (f + 1) * 128],
                        ln1eT[:, c, q0 : q0 + qn],
                        start=(c == 0), stop=(c == 3),
                    )
                if on_act:
                    nc.scalar.activation(
                        out=QT[:, f, q0 : q0 + qn], in_=ps[:, :qn],
                        func=AF.Identity, bias=qkvbc[:, 4 + f : 5 + f], scale=1.0,
                    )
                else:
                    nc.vector.tensor_scalar(
                        out=QT[:, f, q0 : q0 + qn], in0=ps[:, :qn],
                        scalar1=qkvbc[:, 4 + f : 5 + f], scalar2=None,
                        op0=OP.add,
                    )

            # ext LN1 + qc1's Q blocks first: attention pr0 (qc1) starts as
            # soon as K/V chunk 0 lands.
            for t0, ts in etiles[4:7]:
                ln1_tile(xe_d, t0, ts, put_ln1e)
            q_proj_f(0, *QCHS[1])
            for t0, ts in etiles[0:4]:
                ln1_tile(xe_d, t0, ts, put_ln1e)
            for f in range(1, 4):
                q_proj_f(f, *QCHS[1])

            # ---------------- attention ----------------
            # Per (qc, pr): stream of 50 score-matmul outputs (kt-major,
            # head A then B) packed into 3-bank psum group tiles; one fused
            # exp per group; PV matmuls consume the bf16 exp output.
            oTs = big.tile([64, 8, EXT], b16, tag="oTs")
            a_sb = big.tile([128, 7, C], b16, tag="a_sb")
            # reuses ln1eT's slot — dead after the Q projection
            ln2aT = big.tile([128, 4, EXT], b16, tag="le")

            def put_ln2a(cc, t0, ts):
                return ln2aT[:, cc, t0 : t0 + ts]

            def outproj_etile(i, t0, ts):
                # out-proj + residual only; LN2 happens post-attention so no
                # ACT table switch lands inside the exp window.  out_b is
                # pre-added to xeb on the host.
                ps = mk_ps()
                for h in range(8):
                    nc.tensor.matmul(
                        ps, oTs[:, h, t0 : t0 + ts], outTs[:, h, :],
                        start=(h == 0), stop=(h == 7),
                    )
                xt = stage.tile([128, C], f32, tag="xf")
                nc.gpsimd.dma_start(out=xt[:ts], in_=xeb_d[t0 : t0 + ts, :])
                nc.vector.tensor_add(out=a_sb[:ts, i, :], in0=xt[:ts], in1=ps[:ts])

            def ln2_etile(i, t0, ts):
                lt = stage.tile([128, C], b16, tag="xl")
                layer_norm_tile(a_sb[:, i, :], ts, lt)
                dma_transpose(lt, ts, t0, put_ln2a)

            # softmax-sum staging for the deferred normalize
            srows = big.tile([1, 8, 512], b16, tag="srows")
            pending = []  # (slot, head, q0, qn) awaiting recip+broadcast+norm

            def emit_recips(todo):
                if not todo:
                    return
                slot0 = todo[0][0]
                ns = len(todo)
                qn = todo[0][3]
                nq = qn // 128
                # reshape the 1-lane sum rows onto 128 lanes via DRAM,
                # reciprocal there (~nq elems/lane), write back
                nc.sync.dma_start(
                    out=sums_d[slot0 : slot0 + ns, 0:qn],
                    in_=srows[0:1, slot0 : slot0 + ns, 0:qn],
                )
                sv = small.tile([128, 2, 4], b16, tag="sv")
                rv = small.tile([128, 2, 4], b16, tag="rv")
                nc.sync.dma_start(
                    out=sv[:, 0:ns, 0:nq],
                    in_=sums_d[slot0 : slot0 + ns, 0:qn].rearrange(
                        "s (p e) -> p s e", e=nq
                    ),
                )
                with nc.allow_low_precision(reason="1/softmax-sum bf16"):
                    nc.vector.reciprocal(out=rv[:, 0:ns, 0:nq], in_=sv[:, 0:ns, 0:nq])
                nc.sync.dma_start(
                    out=rsum_d[slot0 : slot0 + ns, 0:qn].rearrange(
                        "s (p e) -> p s e", e=nq
                    ),
                    in_=rv[:, 0:ns, 0:nq],
                )

            def emit_norms(todo):
                for slot, h, q0, qn in todo:
                    rbb = stage.tile([64, 512], b16, tag="rbb")
                    nc.sync.dma_start(
                        out=rbb[:, 0:qn],
                        in_=bass.AP(
                            tensor=rsum_d[:, :].tensor, offset=slot * 512,
                            ap=[[0, 64], [1, qn]],
                        ),
                    )
                    dst = oTs[:, h, q0 : q0 + qn]
                    nc.vector.scalar_tensor_tensor(
                        out=dst, in0=dst, scalar=1.0, in1=rbb[:, 0:qn],
                        op0=OP.bypass, op1=OP.mult,
                    )

            nkt = len(btiles)
            njs = 2 * nkt
            ngrp = (njs + 2) // 3
            pending_op = []  # etile indices awaiting out-proj emission

            class PrEmitter:
                """Incrementally emits one (qc, pr)'s attention: score
                groups, fused exps, PV matmuls, plus per-group hooks."""

                def __init__(self, pr, q0, qn, hooks=None):
                    self.pr, self.q0, self.qn = pr, q0, qn
                    self.hooks = hooks or {}
                    self.oA = pso.tile([128, 512], f32, tag="oA", name="oA")
                    self.oB = pso.tile([128, 512], f32, tag="oB", name="oB")
                    self.g = 0
                    self.grp_tiles = {}
                    self.ex_tiles = {}

                def emit_upto(self, gmax):
                    pr, q0, qn = self.pr, self.q0, self.qn
                    while self.g < min(gmax, ngrp):
                        g = self.g
                        js = list(range(3 * g, min(3 * g + 3, njs)))
                        for j in js:
                            kt, hb = j // 2, j % 2
                            k0, kn = btiles[kt]
                            if j % 3 == 0:
                                self.grp_tiles[g] = psg.tile(
                                    [128, 1536], f32, tag="sg", name="sg",
                                )
                            gt = self.grp_tiles[j // 3]
                            col = (j % 3) * 512
                            nc.tensor.matmul(
                                gt[0:kn, col : col + qn],
                                KT[pr][64 * hb : 64 * hb + 64, k0 : k0 + kn],
                                QT[64 * hb : 64 * hb + 64, pr, q0 : q0 + qn],
                                start=True, stop=True,
                                tile_position=(64 * hb, 0),
                            )
                        gt = self.grp_tiles[g]
                        nj = len(js)
                        ex = exr.tile([128, 1536], b16, tag="ex", name="ex")
                        self.ex_tiles[g] = ex
                        gv = gt.rearrange("p (s c) -> p s c", c=512)[:, 0:nj, 0:qn]
                        xv = ex.rearrange("p (s c) -> p s c", c=512)[:, 0:nj, 0:qn]
                        nc.scalar.activation(out=xv, in_=gv, func=AF.Exp)
                        for j in js:
                            kt, hb = j // 2, j % 2
                            k0, kn = btiles[kt]
                            col = (j % 3) * 512
                            o = self.oB if hb else self.oA
                            nc.tensor.matmul(
                                o[0:65, 0:qn],
                                V5[:kn, kt, 2 * pr + hb, :],
                                self.ex_tiles[j // 3][0:kn, col : col + qn],
                                start=(kt == 0), stop=(kt == nkt - 1),
                            )
                        for fn in self.hooks.get(g, ()):
                            fn()
                        self.g += 1

                def finish(self):
                    self.emit_upto(ngrp)
                    pr, q0, qn = self.pr, self.q0, self.qn
                    # evacuate o and the exp-sums with fast DVE copies;
                    # everything slow is deferred to the next pr's hooks.
                    for hb, o in ((0, self.oA), (1, self.oB)):
                        h = 2 * pr + hb
                        slot = pr * 2 + hb
                        nc.vector.tensor_copy(
                            out=oTs[:, h, q0 : q0 + qn], in_=o[0:64, 0:qn]
                        )
                        nc.vector.tensor_copy(
                            out=srows[0:1, slot, 0:qn], in_=o[64:65, 0:qn]
                        )
                        pending.append((slot, h, q0, qn))

            def take_pending_hooks():
                todo = pending[:]
                del pending[:]
                hooks = {
                    7: [lambda t=todo: emit_recips(t)],
                    11: [lambda t=todo: emit_norms(t)],
                }

                def pop_op():
                    if pending_op:
                        ei = pending_op.pop(0)
                        outproj_etile(ei, *etiles[ei])

                hooks[14] = [pop_op]
                return hooks

            # ---- merged head + attention(qc1, pr0): LN1/K/V feed the pr0
            # score groups just-in-time, chunk by chunk ----
            q0_1, qn_1 = QCHS[1]
            st0 = PrEmitter(0, q0_1, qn_1)
            for ci, (c0, cn) in enumerate(bchunks):
                in_chunk = [
                    (i, t0, ts) for i, (t0, ts) in enumerate(btiles)
                    if c0 <= t0 < c0 + cn
                ]
                for i, t0, ts in in_chunk:
                    ln1_tile(xb_d, t0, ts, put_ln1x, i)
                for f in range(4):
                    k_proj_f(f, c0, cn)
                for i, t0, ts in in_chunk:
                    v_proj_tile(i, t0, ts)
                st0.emit_upto((8 * ci + 5) // 3 + 1)
            st0.finish()

            # remaining qc1 prs; qc0's Q blocks slip into pr1's window (DVE
            # copies — the ACT is the gate there)
            for pr in (1, 2, 3):
                hooks = take_pending_hooks()
                if pr == 1:
                    for k in range(4):
                        hooks.setdefault(2 + 4 * k, []).append(
                            lambda f=k: q_proj_f(f, QCHS[0][0], QCHS[0][1],
                                                 on_act=False)
                        )
                PrEmitter(pr, q0_1, qn_1, hooks).finish()

            # qc0 prs; qc1's out-proj etiles fill the windows
            q0_0, qn_0 = QCHS[0]
            for pr in range(4):
                hooks = take_pending_hooks()
                PrEmitter(pr, q0_0, qn_0, hooks).finish()
                if pr < 3:
                    pending_op.append(4 + pr)

            emit_recips(pending)
            emit_norms(pending)
            del pending[:]
            for ei in pending_op:
                outproj_etile(ei, *etiles[ei])
            del pending_op[:]
            for i in range(4):
                outproj_etile(i, *etiles[i])
                ln2_etile(i, *etiles[i])
            for i in range(4, 7):
                ln2_etile(i, *etiles[i])

            # ---------------- MLP: fc1 -> scatter -> PE dwconv -> gelu ----------------
            # fc2 weights arrive late, into slots KT vacated after attention
            fc2Ta = big.tile([128, 8, C], b16, tag="kt0")
            nc.gpsimd.dma_start(
                out=fc2Ta, in_=fc2T_d[0:1024, :].rearrange("(g p) f -> p g f", p=128)
            )
            fc2Tb = big.tile([128, 8, C], b16, tag="kt1")
            nc.gpsimd.dma_start(
                out=fc2Tb, in_=fc2T_d[1024:2048, :].rearrange("(g p) f -> p g f", p=128)
            )
            ghT = [big.tile([128, 4, OWN], b16, tag=f"lx{k}", name=f"ghT{k}") for k in range(4)]
            SPAN = RPC * (WI + 2)          # 812 flat conv span (2 junk cols/row)
            HSPAN = SPAN // 2              # 406 = 7 rows
            PADW = EXTR * (WI + 2) + 2     # 930: +2 guard for last-tap reads
            for g in range(16):
                pad = padp.tile([128, PADW], b16, tag="pad")
                padv = pad[:, : PADW - 2].rearrange("p (r x) -> p r x", x=WI + 2)
                nc.vector.memset(pad[:, PADW - 2 :], 0.0)
                nc.vector.memset(padv[:, :, 0:1], 0.0)
                nc.vector.memset(padv[:, :, WI + 1 : WI + 2], 0.0)
                for fi, (f0, fn) in enumerate(FCHS):
                    ps = mk_ps()
                    for c in range(4):
                        nc.tensor.matmul(
                            ps[:, :fn],
                            fc1T[:, c, g * 128 : (g + 1) * 128],
                            ln2aT[:, c, f0 : f0 + fn],
                            start=(c == 0), stop=(c == 3),
                        )
                    r0 = f0 // WI
                    nr = fn // WI
                    # (fc1 + bias) * mask in one STT (bias is per-partition)
                    nc.vector.scalar_tensor_tensor(
                        out=padv[:, r0 : r0 + nr, 1 : WI + 1],
                        in0=ps[:, :fn].rearrange("p (r x) -> p r x", x=WI),
                        scalar=fc1bg[:, g : g + 1],
                        in1=maskb[:, f0 : f0 + fn].rearrange("p (r x) -> p r x", x=WI),
                        op0=OP.add, op1=OP.mult,
                    )
                # diagonal weight matrices for this group's 9 taps
                dg = dgp.tile([128, 9, 128], b16, tag="dg")
                for tap in range(9):
                    nc.vector.tensor_scalar_mul(
                        out=dg[:, tap, :], in0=ident, scalar1=dww[:, g, tap : tap + 1]
                    )
                # 3x3 depthwise conv: 9 accumulating diag matmuls per chunk
                for ch in range(2):
                    cps = psg.tile([128, 1536], f32, tag="sg", name="cps")
                    for dy in range(3):
                        for dx in range(3):
                            tap = 3 * dy + dx
                            off = dy * (WI + 2) + dx + ch * HSPAN
                            nc.tensor.matmul(
                                cps[:, 0:HSPAN],
                                dg[:, tap, :],
                                pad[:, off : off + HSPAN],
                                start=(tap == 0), stop=(tap == 8),
                            )
                    # gelu(conv + dwb) straight out of PSUM, skipping the
                    # 2 junk cols per row
                    cv = cps[:, 0:HSPAN].rearrange("p (r x) -> p r x", x=WI + 2)
                    nc.scalar.activation(
                        out=ghT[g // 4][:, g % 4, ch * 392 : (ch + 1) * 392],
                        in_=cv[:, :, 0:WI],
                        func=AF.Gelu, bias=dwb[:, g : g + 1], scale=1.0,
                    )

            # ---------------- fc2 + final residual ----------------
            for i, (t0, ts) in enumerate(otiles):
                ps = mk_ps()
                for k in range(16):
                    f2 = fc2Ta[:, k, :] if k < 8 else fc2Tb[:, k - 8, :]
                    nc.tensor.matmul(
                        ps[:ts],
                        ghT[k // 4][:, k % 4, t0 : t0 + ts],
                        f2,
                        start=(k == 0), stop=False,
                    )
                nc.tensor.matmul(ps[:ts], ones[:, :ts], fc2b, start=False, stop=True)
                at = stage.tile([128, C], f32, tag="xf")
                n1 = min(ts, 128 - WI)  # rows from a tile i (partitions WI..)
                nc.gpsimd.dma_start(out=at[:n1], in_=a_sb[WI : WI + n1, i, :])
                if ts > n1:
                    nc.gpsimd.dma_start(
                        out=at[n1:ts], in_=a_sb[0 : ts - n1, i + 1, :]
                    )
                ot = stage.tile([128, C], f32, tag="xa")
                nc.vector.tensor_add(out=ot[:ts], in0=at[:ts], in1=ps[:ts])
                nc.gpsimd.dma_start(out=out_d[t0 : t0 + ts, :], in_=ot[:ts])

    return nc


def _prep_host(inputs):
    import ml_dtypes

    bf16 = ml_dtypes.bfloat16
    f32 = np.float32

    g = {k: np.asarray(v) for k, v in inputs.items()}
    x = g["x"].astype(f32)
    ln1_w, ln1_b = g["ln1_w"].astype(f32), g["ln1_b"].astype(f32)
    ln2_w, ln2_b = g["ln2_w"].astype(f32), g["ln2_b"].astype(f32)
    qkv_w, qkv_b = g["qkv_w"].astype(f32), g["qkv_b"].astype(f32)
    out_w, out_b = g["out_w"].astype(f32), g["out_b"].astype(f32)
    fc1_w, fc1_b = g["fc1_w"].astype(f32), g["fc1_b"].astype(f32)
    fc2_w, fc2_b = g["fc2_w"].astype(f32), g["fc2_b"].astype(f32)
    dw_w, dw_b = g["dw_w"].astype(f32), g["dw_b"].astype(f32)
    temp = float(np.asarray(g["temperature"]))

    # fold LN affine into the following matmul; fold 1/temperature into W_q
    qkv_w2 = qkv_w * ln1_w[None, :]
    qkv_b2 = qkv_b + qkv_w @ ln1_b
    qkv_w2[:C] /= temp
    qkv_b2[:C] /= temp
    fc1_w2 = fc1_w * ln2_w[None, :]
    fc1_b2 = fc1_b + fc1_w @ ln2_b

    # biases repacked for on-engine folding:
    #   qkvbc[:, f]   = K bias for feature block f (per partition)
    #   qkvbc[:, 4+f] = Q bias for feature block f
    #   vbias         = V bias row (broadcast-added during the V5 copy)
    #   fc1bg[p, g]   = fc1 bias of hidden unit g*128+p
    qkvbc = np.zeros((128, 8), f32)
    for f in range(4):
        qkvbc[:, f] = qkv_b2[C + f * 128 : C + (f + 1) * 128]
        qkvbc[:, 4 + f] = qkv_b2[f * 128 : (f + 1) * 128]
    shared = {
        "qkvT": np.ascontiguousarray(qkv_w2.T).astype(bf16),
        "qkvbc": qkvbc,
        "vbias": np.ascontiguousarray(qkv_b2[None, 2 * C : 3 * C]).astype(f32),
        "outT": np.ascontiguousarray(out_w.T).astype(bf16),
        "fc1T": np.ascontiguousarray(fc1_w2.T).astype(bf16),
        "fc1bg": np.ascontiguousarray(fc1_b2.reshape(16, 128).T).astype(f32),
        "fc2T": np.ascontiguousarray(fc2_w.T).astype(bf16),
        "fc2b": fc2_b[None, :].astype(bf16),
        "dww": np.ascontiguousarray(dw_w.reshape(HID, 9)).astype(f32),
        "dwb": dw_b.astype(f32),
        "ident": np.eye(128, dtype=f32).astype(bf16),
    }

    ximg = x.reshape(B, HI, WI, C)
    in_maps = []
    for c in range(NCORES):
        b, qi = c // 4, c % 4
        r0 = RPC * qi
        xe = np.zeros((EXTR, WI, C), f32)
        mask = np.zeros((EXTR, WI), f32)
        for e in range(EXTR):
            r = r0 - 1 + e
            if 0 <= r < HI:
                xe[e] = ximg[b, r]
                mask[e] = 1.0
        m = dict(shared)
        m["xb"] = np.ascontiguousarray(x[b])
        m["xe"] = np.ascontiguousarray(xe.reshape(EXT, C))
        m["xeb"] = np.ascontiguousarray(xe.reshape(EXT, C) + out_b[None, :])
        m["mask"] = mask.reshape(EXT).astype(bf16)
        in_maps.append(m)
    return in_maps


def _run(inputs, trace=False):
    from concourse.bass_utils import run_bass_kernel_spmd

    if "nc" not in _CACHE:
        nc = _build_nc()
        nc.finalize()
        _CACHE["nc"] = nc
    nc = _CACHE["nc"]
    in_maps = _prep_host(inputs)
    res = run_bass_kernel_spmd(nc, in_maps, core_ids=list(range(NCORES)), trace=trace)

    x = np.asarray(inputs["x"])
    out = np.zeros((B, NB, C), np.float32)
    for c in range(NCORES):
        b, qi = c // 4, c % 4
        r0 = RPC * qi
        out[b, r0 * WI : (r0 + RPC) * WI, :] = res.results[c]["out"]
    return out.astype(x.dtype, copy=False), res


def kernel(**inputs) -> np.ndarray:
    out, _ = _run(inputs, trace=False)
    return out
